# revision 1
# baseline (speedup 1.0000x reference)
"""Longformer sliding-window self-attention (MBart variant) on 8 TRN2 cores.

Strategy: sequence-parallel sharding. Each of the 8 cores gets one
(batch, quarter-sequence) shard: core c -> batch c//4, queries
[1024*(c%4), 1024*(c%4+1)). Each core receives a halo'd slice of the
hidden states (1536 rows, transposed, zero-padded at sequence edges),
computes Q/K/V projections, banded attention over 4 chunks of 256
queries x 768-key windows, and the output projection, returning its
[768, 1024] transposed output slice. Host re-assembles the full
[2, 4096, 768] output.

All matmuls run as float32r (full PE rate, ~1e-4 component error).

Math notes (exact rewrites of the reference):
  - query scale 1/sqrt(64) folded into Wq/bq on host.
  - Wk bias drops out of softmax entirely (constant per query row).
  - Wv bias commutes through softmax (weights sum to 1) and the output
    projection: folded into bo_eff = bo + Wo @ bv on host.
  - band + sequence-edge + attention_mask key bias folded into additive
    per-core mask tiles built on host (NEG = -1e9 outside the band).
  - softmax computed unnormalized; the denominator is produced by an
    extra all-ones column appended to each head's V block, and applied
    as a reciprocal multiply on the context rows.
"""

import numpy as np

# problem shapes (fixed by the task)
B, S, D, H = 2, 4096, 768, 12
DH = D // H            # 64
W = 256                # one-sided window == chunk size b
NEG = -1e9
NCORES = 8
G = 4                  # sequence groups per batch (8 cores / 2 batches)
SLOC = S // G          # 1024 queries per core
SH = SLOC + 2 * W      # 1536 halo'd rows per core
NB = SLOC // W         # 4 chunks per core
NT = 3 * W // 128      # 6 key tiles of 128 per chunk window
P = 128
DJ = D // P            # 6 tiles of 128 over the model dim

_PROGRAM_CACHE: dict = {}


def _build_program(general_mask: bool):
    import concourse.bacc as bacc
    import concourse.mybir as mybir
    import concourse.tile as tile
    from contextlib import ExitStack

    F32 = mybir.dt.float32
    F32R = mybir.dt.float32r
    AF = mybir.ActivationFunctionType
    NS = 3 if general_mask else 2        # mask slots per chunk
    MCOLS = NB * NS * 512                # mask sbuf columns

    nc = bacc.Bacc("TRN2", target_bir_lowering=False, debug=False)

    hsT = nc.dram_tensor("hsT", [D, SH], F32R, kind="ExternalInput")
    wqT = nc.dram_tensor("wqT", [D, D], F32R, kind="ExternalInput")
    wkT = nc.dram_tensor("wkT", [D, D], F32R, kind="ExternalInput")
    wvT = nc.dram_tensor("wvT", [D, D], F32R, kind="ExternalInput")
    woT = nc.dram_tensor("woT", [D, D], F32R, kind="ExternalInput")
    bq = nc.dram_tensor("bq", [D], F32, kind="ExternalInput")
    boe = nc.dram_tensor("boe", [D], F32, kind="ExternalInput")
    masks = nc.dram_tensor("masks", [NB, NS, P, 512], F32, kind="ExternalInput")
    outT = nc.dram_tensor("outT", [D, SLOC], F32, kind="ExternalOutput")

    with tile.TileContext(nc) as tc, ExitStack() as stack:
        const = stack.enter_context(tc.tile_pool(name="const", bufs=1))
        qt_p = stack.enter_context(tc.tile_pool(name="qt", bufs=1))
        kt_p = stack.enter_context(tc.tile_pool(name="kt", bufs=1))
        ct_p = stack.enter_context(tc.tile_pool(name="ct", bufs=1))

        bq_sb = const.tile([P, DJ], F32, tag="bq")
        nc.sync.dma_start(out=bq_sb[:], in_=bq.rearrange("(t p) -> p t", p=P))
        boe_sb = const.tile([P, DJ], F32, tag="boe")
        nc.sync.dma_start(out=boe_sb[:], in_=boe.rearrange("(t p) -> p t", p=P))
        mask_sb = const.tile([P, MCOLS], F32, tag="masks")
        for n in range(NB):
            for sl in range(NS):
                off = (n * NS + sl) * 512
                nc.sync.dma_start(
                    out=mask_sb[:, off : off + 512], in_=masks[n, sl]
                )

        QT = [qt_p.tile([P, SLOC], F32R, tag=f"qt{j}", name=f"qt{j}") for j in range(DJ)]
        KT = [kt_p.tile([P, SH], F32R, tag=f"kt{j}", name=f"kt{j}") for j in range(DJ)]
        CT = [ct_p.tile([P, SLOC], F32R, tag=f"ct{j}", name=f"ct{j}") for j in range(DJ)]

        # ---------------- phase 1: projections ------------------------
        # pool stack is LIFO: va (persistent) must open before hs.
        va_p = stack.enter_context(tc.tile_pool(name="va", bufs=1))
        VA = [va_p.tile([P, H * (DH + 1)], F32R, tag=f"va{s}", name=f"va{s}") for s in range(SH // P)]

        ones_f = const.tile([P, H], F32, tag="ones_f")
        nc.vector.memset(ones_f[:], 1.0)
        for st in range(SH // P):
            view = VA[st].rearrange("p (h e) -> p h e", e=DH + 1)
            nc.vector.tensor_copy(view[:, :, DH : DH + 1], ones_f[:])

        hs_stack = ExitStack()
        hs_p = hs_stack.enter_context(tc.tile_pool(name="hs", bufs=1))
        HS = [hs_p.tile([P, SH], F32R, tag=f"hs{i}", name=f"hs{i}") for i in range(DJ)]
        for i in range(DJ):
            nc.sync.dma_start(out=HS[i][:], in_=hsT[P * i : P * (i + 1), :])

        # V = hs @ Wv.T  -> VA[st] tiles [128 seq, 768 head-dims]
        with (
            tc.tile_pool(name="wv", bufs=1) as wv_p,
            tc.tile_pool(name="ps2", bufs=4, space="PSUM") as ps2,
        ):
            for half in range(2):
                WV = [wv_p.tile([P, 384], F32R, tag=f"wv{i}", name=f"wv{i}") for i in range(DJ)]
                for i in range(DJ):
                    nc.sync.dma_start(
                        out=WV[i][:],
                        in_=wvT[P * i : P * (i + 1), 384 * half : 384 * (half + 1)],
                    )
                for st in range(SH // P):
                    ps = ps2.tile([P, 384], F32, tag="ps2")
                    for i in range(DJ):
                        nc.tensor.matmul(
                            ps[:],
                            HS[i][:, P * st : P * (st + 1)],
                            WV[i][:],
                            start=(i == 0),
                            stop=(i == DJ - 1),
                        )
                    view = VA[st].rearrange("p (h e) -> p h e", e=DH + 1)
                    nc.vector.tensor_copy(
                        view[:, 6 * half : 6 * (half + 1), 0:DH],
                        ps[:].rearrange("p (h e) -> p h e", e=DH),
                    )

        # QT[j] = (Wq/8) @ hs_loc.T + bq/8
        with (
            tc.tile_pool(name="wq", bufs=1) as wq_p,
            tc.tile_pool(name="ps1", bufs=4, space="PSUM") as ps1,
        ):
            for half in range(2):
                WQ = [wq_p.tile([P, 384], F32R, tag=f"wq{i}", name=f"wq{i}") for i in range(DJ)]
                for i in range(DJ):
                    nc.sync.dma_start(
                        out=WQ[i][:],
                        in_=wqT[P * i : P * (i + 1), 384 * half : 384 * (half + 1)],
                    )
                for j in range(3 * half, 3 * half + 3):
                    jc = P * j - 384 * half
                    for sp in range(SLOC // 512):
                        ps = ps1.tile([P, 512], F32, tag="ps1")
                        for i in range(DJ):
                            nc.tensor.matmul(
                                ps[:],
                                WQ[i][:, jc : jc + P],
                                HS[i][:, W + 512 * sp : W + 512 * (sp + 1)],
                                start=(i == 0),
                                stop=(i == DJ - 1),
                            )
                        nc.scalar.activation(
                            QT[j][:, 512 * sp : 512 * (sp + 1)],
                            ps[:],
                            AF.Identity,
                            bias=bq_sb[:, j : j + 1],
                        )

        # KT[j] = Wk @ hs_halo.T  (bias bk cancels in softmax)
        with (
            tc.tile_pool(name="wk", bufs=1) as wk_p,
            tc.tile_pool(name="ps1b", bufs=4, space="PSUM") as ps1b,
        ):
            for half in range(2):
                WK = [wk_p.tile([P, 384], F32R, tag=f"wk{i}", name=f"wk{i}") for i in range(DJ)]
                for i in range(DJ):
                    nc.sync.dma_start(
                        out=WK[i][:],
                        in_=wkT[P * i : P * (i + 1), 384 * half : 384 * (half + 1)],
                    )
                for j in range(3 * half, 3 * half + 3):
                    jc = P * j - 384 * half
                    for sp in range(SH // 512):
                        ps = ps1b.tile([P, 512], F32, tag="ps1b")
                        for i in range(DJ):
                            nc.tensor.matmul(
                                ps[:],
                                WK[i][:, jc : jc + P],
                                HS[i][:, 512 * sp : 512 * (sp + 1)],
                                start=(i == 0),
                                stop=(i == DJ - 1),
                            )
                        nc.scalar.activation(
                            KT[j][:, 512 * sp : 512 * (sp + 1)], ps[:], AF.Copy
                        )
        hs_stack.close()

        # ---------------- phase 2: banded attention -------------------
        with (
            tc.tile_pool(name="expp", bufs=3) as exp_p,
            tc.tile_pool(name="dn", bufs=4) as dn_p,
            tc.tile_pool(name="pss", bufs=2, space="PSUM") as pss,
            tc.tile_pool(name="psc", bufs=2, space="PSUM") as psc,
        ):
            for n in range(NB):
                for j in range(DJ):
                    sps = [pss.tile([P, NT * W], F32, tag="s", name=f"s{n}_{j}_{k}") for k in range(2)]
                    for t in range(NT):
                        for hh in range(2):
                            r0 = DH * hh
                            nc.tensor.matmul(
                                sps[hh][:, W * t : W * (t + 1)],
                                KT[j][r0 : r0 + DH, W * n + P * t : W * n + P * (t + 1)],
                                QT[j][r0 : r0 + DH, W * n : W * (n + 1)],
                                start=True,
                                stop=True,
                            )
                    for hh in range(2):
                        h = 2 * j + hh
                        moff = n * NS * 512
                        nc.vector.tensor_add(
                            sps[hh][:, 0:512],
                            sps[hh][:, 0:512],
                            mask_sb[:, moff : moff + 512],
                        )
                        if general_mask:
                            nc.vector.tensor_add(
                                sps[hh][:, 512:1024],
                                sps[hh][:, 512:1024],
                                mask_sb[:, moff + 512 : moff + 1024],
                            )
                        lastoff = moff + (NS - 1) * 512
                        nc.vector.tensor_add(
                            sps[hh][:, 1024:1536],
                            sps[hh][:, 1024:1536],
                            mask_sb[:, lastoff : lastoff + 512],
                        )
                        expt = exp_p.tile([P, NT * W], F32R, tag="e", name=f"e{n}_{j}_{hh}")
                        nc.scalar.activation(expt[:], sps[hh][:], AF.Exp)
                        cps = psc.tile([DH + 1, W], F32, tag="c", name=f"c{n}_{h}")
                        for t in range(NT):
                            nc.tensor.matmul(
                                cps[:],
                                VA[2 * n + t][:, (DH + 1) * h : (DH + 1) * (h + 1)],
                                expt[:, W * t : W * (t + 1)],
                                start=(t == 0),
                                stop=(t == NT - 1),
                            )
                        # denominator row -> partition 0 (DMA hop), reciprocal,
                        # broadcast over the head's 64 lanes, fused normalize
                        dcp = dn_p.tile([DH + 1, W], F32, tag="dcp", name=f"dcp{n}_{h}")
                        nc.vector.tensor_copy(dcp[DH : DH + 1, :], cps[DH : DH + 1, :])
                        dnrow = dn_p.tile([1, W], F32, tag="dnr", name=f"dnr{n}_{h}")
                        nc.sync.dma_start(out=dnrow[:], in_=dcp[DH : DH + 1, :])
                        rcrow = dn_p.tile([1, W], F32, tag="rcr", name=f"rcr{n}_{h}")
                        scr = dn_p.tile([1, W], F32, tag="scr", name=f"scr{n}_{h}")
                        nc.vector.reciprocal_approx_accurate(
                            out=rcrow[:], in_=dnrow[:], scratch=scr[:]
                        )
                        rb = dn_p.tile([DH, W], F32, tag="rb", name=f"rb{n}_{h}")
                        nc.gpsimd.partition_broadcast(rb[:], rcrow[:], channels=DH)
                        if hh == 0:
                            nc.vector.tensor_mul(
                                CT[j][0:DH, W * n : W * (n + 1)], cps[0:DH, :], rb[:]
                            )
                        else:
                            stg = dn_p.tile([DH, W], F32R, tag="stg", name=f"stg{n}_{h}")
                            nc.vector.tensor_mul(stg[:], cps[0:DH, :], rb[:])
                            nc.sync.dma_start(
                                out=CT[j][DH:P, W * n : W * (n + 1)], in_=stg[:]
                            )

        # ---------------- phase 3: output projection ------------------
        with (
            tc.tile_pool(name="wo", bufs=1) as wo_p,
            tc.tile_pool(name="ob", bufs=3) as ob_p,
            tc.tile_pool(name="ps3", bufs=4, space="PSUM") as ps3,
        ):
            WO = [wo_p.tile([P, D], F32R, tag=f"wo{i}", name=f"wo{i}") for i in range(DJ)]
            for i in range(DJ):
                nc.sync.dma_start(out=WO[i][:], in_=woT[P * i : P * (i + 1), :])
            for j in range(DJ):
                for sp in range(SLOC // 512):
                    ps = ps3.tile([P, 512], F32, tag="ps3")
                    for i in range(DJ):
                        nc.tensor.matmul(
                            ps[:],
                            WO[i][:, P * j : P * (j + 1)],
                            CT[i][:, 512 * sp : 512 * (sp + 1)],
                            start=(i == 0),
                            stop=(i == DJ - 1),
                        )
                    osb = ob_p.tile([P, 512], F32, tag="ob")
                    nc.scalar.activation(
                        osb[:], ps[:], AF.Identity, bias=boe_sb[:, j : j + 1]
                    )
                    nc.sync.dma_start(
                        out=outT[P * j : P * (j + 1), 512 * sp : 512 * (sp + 1)],
                        in_=osb[:],
                    )

    nc.compile()
    return nc


def _host_prep(hidden_states, attention_mask, Wq, bq, Wk, bk, Wv, bv, Wo, bo):
    """Build per-core input maps. Returns (in_maps, general_mask)."""
    hs = np.asarray(hidden_states, dtype=np.float32)
    am = np.asarray(attention_mask, dtype=np.float32)
    Wq = np.asarray(Wq, dtype=np.float32)
    Wk = np.asarray(Wk, dtype=np.float32)
    Wv = np.asarray(Wv, dtype=np.float32)
    Wo = np.asarray(Wo, dtype=np.float32)
    bq = np.asarray(bq, dtype=np.float32)
    bv = np.asarray(bv, dtype=np.float32)
    bo = np.asarray(bo, dtype=np.float32)

    general = bool(np.any(am != 0.0))
    NS = 3 if general else 2
    scale = 1.0 / np.sqrt(np.float32(DH))

    wqT = np.ascontiguousarray(Wq.T * scale)
    wkT = np.ascontiguousarray(Wk.T)
    wvT = np.ascontiguousarray(Wv.T)
    woT = np.ascontiguousarray(Wo.T)
    bq_s = (bq * scale).astype(np.float32)
    bo_eff = (bo + Wo @ bv).astype(np.float32)

    # band validity per (tile t, partition p, q): kpos_w = 128 t + p
    t_idx = np.arange(NT)[:, None, None]
    p_idx = np.arange(P)[None, :, None]
    q_idx = np.arange(W)[None, None, :]
    kpos_w = P * t_idx + p_idx                      # [6,128,1]
    band_ok = np.abs(kpos_w - W - q_idx) <= W       # [6,128,256]

    in_maps = []
    for c in range(NCORES):
        bi, g = divmod(c, G)
        lo = SLOC * g - W
        halo = np.zeros((SH, D), dtype=np.float32)
        s0, s1 = max(lo, 0), min(lo + SH, S)
        halo[s0 - lo : s1 - lo] = hs[bi, s0:s1]
        hsT_c = np.ascontiguousarray(halo.T)

        m = np.empty((NB, NS, P, 512), dtype=np.float32)
        slot_tiles = [(0, 1), (2, 3), (4, 5)] if general else [(0, 1), (4, 5)]
        for n in range(NB):
            gc = NB * g + n                          # global chunk index
            kglob = W * gc + kpos_w - W              # [6,128,1]
            inb = (kglob >= 0) & (kglob < S)
            if general:
                kb = np.where(
                    inb, -am[bi, np.clip(kglob, 0, S - 1)], 0.0
                )                                    # [6,128,1] key bias
            else:
                kb = np.zeros_like(kglob, dtype=np.float32)
            valid = band_ok & inb
            mt = np.where(valid, kb, NEG).astype(np.float32)  # [6,128,256]
            for sl, (ta, tb) in enumerate(slot_tiles):
                m[n, sl, :, 0:256] = mt[ta]
                m[n, sl, :, 256:512] = mt[tb]

        in_maps.append(
            {
                "hsT": hsT_c,
                "wqT": wqT,
                "wkT": wkT,
                "wvT": wvT,
                "woT": woT,
                "bq": bq_s,
                "boe": bo_eff,
                "masks": m,
            }
        )
    return in_maps, general


def _run(inputs: dict, trace: bool = False):
    """Run the sharded kernel. Returns (full_output, BassKernelResults)."""
    from concourse.bass_utils import run_bass_kernel_spmd

    in_maps, general = _host_prep(**inputs)
    key = ("nc", general)
    if key not in _PROGRAM_CACHE:
        _PROGRAM_CACHE[key] = _build_program(general)
    nc = _PROGRAM_CACHE[key]

    res = run_bass_kernel_spmd(
        nc, in_maps, list(range(NCORES)), trace=trace
    )
    out = np.empty((B, S, D), dtype=np.float32)
    for c in range(NCORES):
        bi, g = divmod(c, G)
        out[bi, SLOC * g : SLOC * (g + 1), :] = res.results[c]["outT"].T
    return out, res


def kernel(**inputs) -> np.ndarray:
    out, _ = _run(inputs, trace=False)
    return out



# revision 2
# speedup vs baseline: 1.5753x; 1.5753x over previous
"""Longformer sliding-window self-attention (MBart variant) on 8 TRN2 cores.

Sequence-parallel sharding: core c -> batch c//4, queries
[1024*(c%4), 1024*(c%4+1)). Each core gets a halo'd, transposed,
bf16 slice of hidden states (1536 rows), computes Q/K/V projections,
banded attention, and the output projection, returning a [768, 1024]
fp32 transposed output slice. Host re-assembles [2, 4096, 768].

v2 design notes (all rewrites are exact up to bf16 rounding of the
inputs, verified ~4e-3 max-rel on the fixed problem seed vs 2e-2 gate):
  - hidden states + all four weight matrices are shipped bf16 (halves
    HBM traffic); query scale folded into Wq/bq, Wk bias cancels in
    softmax, Wv bias commutes (bo_eff = bo + Wo @ bv).
  - banded scores per (chunk n, head h) are trimmed: key tile 0 only
    covers queries 0:128, tile 5 only 128:256 (the rest is out of band)
    -> 1280 score columns instead of 1536, in three [128,<=512] PSUM
    tiles so many (n,h) units pipeline through the 8 PSUM banks.
  - band/edge masks are additive bf16 tiles; interior group G1 needs no
    mask at all when attention_mask == 0.
  - context matmuls run TRANSPOSED: lhsT = exp-tile [keys, queries],
    rhs = V augmented with a ones column [keys, 65] -> PSUM [queries,
    64 ctx + denominator]. Queries on partitions means the softmax
    denominator is a per-partition scalar: one reciprocal [128,1] and
    one tensor_scalar_mul normalize a whole (n, h, q-half) block.
  - normalized [q, e] blocks collect into per-(n, q-half) bf16 slabs
    and a 128x128 bf16 DMA-transpose rebuilds the [D, seq] layout the
    output projection needs.
"""

import numpy as np
import ml_dtypes

# problem shapes (fixed by the task)
B, S, D, H = 2, 4096, 768, 12
DH = D // H            # 64
W = 256                # one-sided window == chunk size b
NEG = -1e9
NCORES = 8
G = 4                  # sequence groups per batch (8 cores / 2 batches)
SLOC = S // G          # 1024 queries per core
SH = SLOC + 2 * W      # 1536 halo'd rows per core
NB = SLOC // W         # 4 chunks per core
P = 128
DJ = D // P            # 6 tiles of 128 over the model dim

_PROGRAM_CACHE: dict = {}

# expt slab column layout per (n, h): (key tile t, q range) -> cols
_EXPT_SEGS = [(0, 0, 128), (1, 0, 256), (2, 0, 256),
              (3, 0, 256), (4, 0, 256), (5, 128, 256)]
# ctx matmul source columns per q-half: list of (t, expt col start)
_CTX_QH = [
    [(0, 0), (1, 128), (2, 384), (3, 640), (4, 896)],          # q 0:128
    [(1, 256), (2, 512), (3, 768), (4, 1024), (5, 1152)],      # q 128:256
]


def _build_program(general_mask: bool):
    import concourse.bacc as bacc
    import concourse.mybir as mybir
    import concourse.tile as tile
    from contextlib import ExitStack

    F32 = mybir.dt.float32
    BF = mybir.dt.bfloat16
    AF = mybir.ActivationFunctionType
    NS = 3 if general_mask else 2      # mask slots per chunk
    MW = 512 if general_mask else 384  # mask tile width

    nc = bacc.Bacc("TRN2", target_bir_lowering=False, debug=False)

    hsT = nc.dram_tensor("hsT", [D, SH], BF, kind="ExternalInput")
    wqT = nc.dram_tensor("wqT", [D, D], BF, kind="ExternalInput")
    wkT = nc.dram_tensor("wkT", [D, D], BF, kind="ExternalInput")
    wvT = nc.dram_tensor("wvT", [D, D], BF, kind="ExternalInput")
    woT = nc.dram_tensor("woT", [D, D], BF, kind="ExternalInput")
    bq = nc.dram_tensor("bq", [D], F32, kind="ExternalInput")
    boe = nc.dram_tensor("boe", [D], F32, kind="ExternalInput")
    masks = nc.dram_tensor("masks", [NB, NS, P, MW], BF, kind="ExternalInput")
    outT = nc.dram_tensor("outT", [D, SLOC], F32, kind="ExternalOutput")

    with tile.TileContext(nc) as tc, ExitStack() as stack:
        const = stack.enter_context(tc.tile_pool(name="const", bufs=1))
        qt_p = stack.enter_context(tc.tile_pool(name="qt", bufs=1))
        kt_p = stack.enter_context(tc.tile_pool(name="kt", bufs=1))
        ct_p = stack.enter_context(tc.tile_pool(name="ct", bufs=1))
        va_p = stack.enter_context(tc.tile_pool(name="va", bufs=1))

        bq_sb = const.tile([P, DJ], F32, tag="bq")
        nc.sync.dma_start(out=bq_sb[:], in_=bq.rearrange("(t p) -> p t", p=P))
        boe_sb = const.tile([P, DJ], F32, tag="boe")
        nc.sync.dma_start(out=boe_sb[:], in_=boe.rearrange("(t p) -> p t", p=P))
        mask_sb = const.tile([P, NB * NS * MW], BF, tag="masks")
        for n in range(NB):
            for sl in range(NS):
                off = (n * NS + sl) * MW
                nc.sync.dma_start(out=mask_sb[:, off:off + MW], in_=masks[n, sl])

        QT = [qt_p.tile([P, SLOC], BF, tag=f"qt{j}", name=f"qt{j}") for j in range(DJ)]
        KT = [kt_p.tile([P, SH], BF, tag=f"kt{j}", name=f"kt{j}") for j in range(DJ)]
        CT = [ct_p.tile([P, SLOC], BF, tag=f"ct{j}", name=f"ct{j}") for j in range(DJ)]
        VA = [va_p.tile([P, H * (DH + 1)], BF, tag=f"va{s}", name=f"va{s}")
              for s in range(SH // P)]
        for st in range(SH // P):
            nc.vector.memset(VA[st][:], 1.0)   # ones columns survive the V copy

        # ---------------- phase 1: projections ------------------------
        hs_stack = ExitStack()
        hs_p = hs_stack.enter_context(tc.tile_pool(name="hs", bufs=1))
        HS = [hs_p.tile([P, SH], BF, tag=f"hs{i}", name=f"hs{i}") for i in range(DJ)]
        for i in range(DJ):
            nc.sync.dma_start(out=HS[i][:], in_=hsT[P * i: P * (i + 1), :])

        # V = hs @ Wv.T -> VA[st] tiles [128 seq, 12*(64+1) head-dims]
        with (
            tc.tile_pool(name="wv", bufs=1) as wv_p,
            tc.tile_pool(name="ps2", bufs=4, space="PSUM") as ps2,
        ):
            WV = [wv_p.tile([P, D], BF, tag=f"wv{i}", name=f"wv{i}") for i in range(DJ)]
            for i in range(DJ):
                nc.sync.dma_start(out=WV[i][:], in_=wvT[P * i: P * (i + 1), :])
            for st in range(SH // P):
                for vh in range(2):
                    ps = ps2.tile([P, 384], F32, tag="ps2")
                    for i in range(DJ):
                        nc.tensor.matmul(
                            ps[:],
                            HS[i][:, P * st: P * (st + 1)],
                            WV[i][:, 384 * vh: 384 * (vh + 1)],
                            start=(i == 0),
                            stop=(i == DJ - 1),
                        )
                    view = VA[st].rearrange("p (h e) -> p h e", e=DH + 1)
                    nc.vector.tensor_copy(
                        view[:, 6 * vh: 6 * (vh + 1), 0:DH],
                        ps[:].rearrange("p (h e) -> p h e", e=DH),
                    )

        # QT[j] = (Wq/8) @ hs_loc.T + bq/8
        with (
            tc.tile_pool(name="wq", bufs=1) as wq_p,
            tc.tile_pool(name="ps1", bufs=4, space="PSUM") as ps1,
        ):
            WQ = [wq_p.tile([P, D], BF, tag=f"wq{i}", name=f"wq{i}") for i in range(DJ)]
            for i in range(DJ):
                nc.sync.dma_start(out=WQ[i][:], in_=wqT[P * i: P * (i + 1), :])
            for j in range(DJ):
                for sp in range(SLOC // 512):
                    ps = ps1.tile([P, 512], F32, tag="ps1")
                    for i in range(DJ):
                        nc.tensor.matmul(
                            ps[:],
                            WQ[i][:, P * j: P * (j + 1)],
                            HS[i][:, W + 512 * sp: W + 512 * (sp + 1)],
                            start=(i == 0),
                            stop=(i == DJ - 1),
                        )
                    nc.scalar.activation(
                        QT[j][:, 512 * sp: 512 * (sp + 1)],
                        ps[:],
                        AF.Identity,
                        bias=bq_sb[:, j: j + 1],
                    )

        # KT[j] = Wk @ hs_halo.T  (bias bk cancels in softmax)
        with (
            tc.tile_pool(name="wk", bufs=1) as wk_p,
            tc.tile_pool(name="ps1b", bufs=4, space="PSUM") as ps1b,
        ):
            WK = [wk_p.tile([P, D], BF, tag=f"wk{i}", name=f"wk{i}") for i in range(DJ)]
            for i in range(DJ):
                nc.sync.dma_start(out=WK[i][:], in_=wkT[P * i: P * (i + 1), :])
            for j in range(DJ):
                for sp in range(SH // 512):
                    ps = ps1b.tile([P, 512], F32, tag="ps1b")
                    for i in range(DJ):
                        nc.tensor.matmul(
                            ps[:],
                            WK[i][:, P * j: P * (j + 1)],
                            HS[i][:, 512 * sp: 512 * (sp + 1)],
                            start=(i == 0),
                            stop=(i == DJ - 1),
                        )
                    nc.scalar.activation(
                        KT[j][:, 512 * sp: 512 * (sp + 1)], ps[:], AF.Copy
                    )
        hs_stack.close()

        # ---------------- phase 2: banded attention -------------------
        # score groups: G0 = t0(q0:128)+t1, G1 = t2+t3, G2 = t4+t5(q128:256)
        with (
            tc.tile_pool(name="expp", bufs=3) as exp_p,
            tc.tile_pool(name="ctx", bufs=1) as ctx_p,
            tc.tile_pool(name="rcp", bufs=6) as rc_p,
            tc.tile_pool(name="pss", bufs=5, space="PSUM") as pss,
            tc.tile_pool(name="psc", bufs=3, space="PSUM") as psc,
        ):
            CX = [ctx_p.tile([P, H * DH], BF, tag=f"cx{u}", name=f"cx{u}")
                  for u in range(2 * NB)]
            for h in range(H):
                jq, r0 = h // 2, DH * (h % 2)
                for n in range(NB):
                    kb = W * n
                    expt = exp_p.tile([P, 1280], BF, tag="e", name=f"e{n}_{h}")
                    # G0: key tiles 0 (q 0:128) and 1 (q 0:256)
                    g0 = pss.tile([P, 512], F32, tag="s", name=f"s0_{n}_{h}")
                    nc.tensor.matmul(
                        g0[:, 0:128],
                        KT[jq][r0:r0 + DH, kb: kb + P],
                        QT[jq][r0:r0 + DH, W * n: W * n + 128],
                        start=True, stop=True,
                    )
                    nc.tensor.matmul(
                        g0[:, 128:384],
                        KT[jq][r0:r0 + DH, kb + P: kb + 2 * P],
                        QT[jq][r0:r0 + DH, W * n: W * (n + 1)],
                        start=True, stop=True,
                    )
                    moff = n * NS * MW
                    nc.vector.tensor_add(
                        g0[:, 0:384], g0[:, 0:384], mask_sb[:, moff: moff + 384]
                    )
                    nc.scalar.activation(expt[:, 0:384], g0[:, 0:384], AF.Exp)
                    # G1: key tiles 2, 3 (fully in-band)
                    g1 = pss.tile([P, 512], F32, tag="s", name=f"s1_{n}_{h}")
                    for u in range(2):
                        nc.tensor.matmul(
                            g1[:, 256 * u: 256 * (u + 1)],
                            KT[jq][r0:r0 + DH, kb + (2 + u) * P: kb + (3 + u) * P],
                            QT[jq][r0:r0 + DH, W * n: W * (n + 1)],
                            start=True, stop=True,
                        )
                    if general_mask:
                        m1 = moff + MW
                        nc.vector.tensor_add(
                            g1[:, 0:512], g1[:, 0:512], mask_sb[:, m1: m1 + 512]
                        )
                    nc.scalar.activation(expt[:, 384:896], g1[:, 0:512], AF.Exp)
                    # G2: key tiles 4 (q 0:256) and 5 (q 128:256)
                    g2 = pss.tile([P, 512], F32, tag="s", name=f"s2_{n}_{h}")
                    nc.tensor.matmul(
                        g2[:, 0:256],
                        KT[jq][r0:r0 + DH, kb + 4 * P: kb + 5 * P],
                        QT[jq][r0:r0 + DH, W * n: W * (n + 1)],
                        start=True, stop=True,
                    )
                    nc.tensor.matmul(
                        g2[:, 256:384],
                        KT[jq][r0:r0 + DH, kb + 5 * P: kb + 6 * P],
                        QT[jq][r0:r0 + DH, W * n + 128: W * (n + 1)],
                        start=True, stop=True,
                    )
                    m2 = moff + (NS - 1) * MW
                    nc.vector.tensor_add(
                        g2[:, 0:384], g2[:, 0:384], mask_sb[:, m2: m2 + 384]
                    )
                    nc.scalar.activation(expt[:, 896:1280], g2[:, 0:384], AF.Exp)
                    # transposed context: [q, 64 ctx + den], queries on partitions
                    for qh in range(2):
                        cps = psc.tile([P, DH + 1], F32, tag="c", name=f"c{n}_{h}_{qh}")
                        segs = _CTX_QH[qh]
                        for si, (t, c0) in enumerate(segs):
                            nc.tensor.matmul(
                                cps[:],
                                expt[:, c0: c0 + P],
                                VA[2 * n + t][:, (DH + 1) * h: (DH + 1) * (h + 1)],
                                start=(si == 0),
                                stop=(si == len(segs) - 1),
                            )
                        rc = rc_p.tile([P, 1], F32, tag="rc", name=f"rc{n}_{h}_{qh}")
                        nc.vector.reciprocal(out=rc[:], in_=cps[:, DH: DH + 1])
                        nc.vector.tensor_scalar_mul(
                            CX[2 * n + qh][:, DH * h: DH * (h + 1)],
                            cps[:, 0:DH],
                            rc[:],
                        )
            # rebuild [D, seq]: 128x128 bf16 DMA transposes
            for n in range(NB):
                for qh in range(2):
                    for j in range(DJ):
                        nc.sync.dma_start_transpose(
                            out=CT[j][:, W * n + P * qh: W * n + P * (qh + 1)],
                            in_=CX[2 * n + qh][:, P * j: P * (j + 1)],
                        )

        # ---------------- phase 3: output projection ------------------
        with (
            tc.tile_pool(name="wo", bufs=1) as wo_p,
            tc.tile_pool(name="ob", bufs=3) as ob_p,
            tc.tile_pool(name="ps3", bufs=4, space="PSUM") as ps3,
        ):
            WO = [wo_p.tile([P, D], BF, tag=f"wo{i}", name=f"wo{i}") for i in range(DJ)]
            for i in range(DJ):
                nc.sync.dma_start(out=WO[i][:], in_=woT[P * i: P * (i + 1), :])
            for j in range(DJ):
                for sp in range(SLOC // 512):
                    ps = ps3.tile([P, 512], F32, tag="ps3")
                    for i in range(DJ):
                        nc.tensor.matmul(
                            ps[:],
                            WO[i][:, P * j: P * (j + 1)],
                            CT[i][:, 512 * sp: 512 * (sp + 1)],
                            start=(i == 0),
                            stop=(i == DJ - 1),
                        )
                    osb = ob_p.tile([P, 512], F32, tag="ob")
                    nc.scalar.activation(
                        osb[:], ps[:], AF.Identity, bias=boe_sb[:, j: j + 1]
                    )
                    nc.sync.dma_start(
                        out=outT[P * j: P * (j + 1), 512 * sp: 512 * (sp + 1)],
                        in_=osb[:],
                    )

    nc.compile()
    return nc


def _band_masks():
    """Fast-path additive band masks, bf16 [128, 384] each."""
    p = np.arange(P)[:, None]
    c = np.arange(384)[None, :]
    # G0: cols 0:128 = t0 (q = c, valid q <= p); 128:384 = t1 (q = c-128,
    # valid q <= p + 128)
    q0 = np.where(c < 128, c, c - 128)
    v0 = np.where(c < 128, q0 <= p, q0 <= p + 128)
    band0 = np.where(v0, 0.0, NEG).astype(ml_dtypes.bfloat16)
    # G2: cols 0:256 = t4 (q = c, valid q >= p); 256:384 = t5 (q = c-128,
    # valid q >= p + 128)
    q2 = np.where(c < 256, c, c - 128)
    v2 = np.where(c < 256, q2 >= p, q2 >= p + 128)
    band2 = np.where(v2, 0.0, NEG).astype(ml_dtypes.bfloat16)
    negm = np.full((P, 384), NEG, dtype=ml_dtypes.bfloat16)
    return band0, band2, negm


def _general_masks(am_row, g):
    """General-path masks [NB, 3, 128, 512] bf16 for one core (batch row
    am_row, sequence group g): band + sequence edge + key bias."""
    out = np.zeros((NB, 3, P, 512), dtype=np.float32)
    p = np.arange(P)[:, None]
    for n in range(NB):
        base = SLOC * g - W + W * n          # key global pos of kpos_w = 0
        for sl in range(3):
            m = out[n, sl]
            if sl == 0:
                segs = [(0, 0, 0, 128), (1, 128, 0, 256)]
            elif sl == 1:
                segs = [(2, 0, 0, 256), (3, 256, 0, 256)]
            else:
                segs = [(4, 0, 0, 256), (5, 256, 128, 256)]
            for t, cstart, qlo, qhi in segs:
                nq = qhi - qlo
                q = (np.arange(nq) + qlo)[None, :]
                kpos = base + t * P + p                     # [128, 1] global
                rel = (t * P + p) - W - q
                band = np.abs(rel) <= W
                inb = (kpos >= 0) & (kpos < S)
                kb = np.where(inb, -am_row[np.clip(kpos, 0, S - 1)], 0.0)
                m[:, cstart:cstart + nq] = np.where(band & inb, kb, NEG)
    return out.astype(ml_dtypes.bfloat16)


def _host_prep(hidden_states, attention_mask, Wq, bq, Wk, bk, Wv, bv, Wo, bo):
    """Build per-core input maps. Returns (in_maps, general_mask)."""
    hs = np.asarray(hidden_states, dtype=np.float32)
    am = np.asarray(attention_mask, dtype=np.float32)
    Wq = np.asarray(Wq, dtype=np.float32)
    Wk = np.asarray(Wk, dtype=np.float32)
    Wv = np.asarray(Wv, dtype=np.float32)
    Wo = np.asarray(Wo, dtype=np.float32)
    bq = np.asarray(bq, dtype=np.float32)
    bv = np.asarray(bv, dtype=np.float32)
    bo = np.asarray(bo, dtype=np.float32)

    general = bool(np.any(am != 0.0))
    scale = 1.0 / np.sqrt(np.float32(DH))

    wqT = np.ascontiguousarray(Wq.T * scale).astype(ml_dtypes.bfloat16)
    wkT = np.ascontiguousarray(Wk.T).astype(ml_dtypes.bfloat16)
    wvT = np.ascontiguousarray(Wv.T).astype(ml_dtypes.bfloat16)
    woT = np.ascontiguousarray(Wo.T).astype(ml_dtypes.bfloat16)
    bq_s = (bq * scale).astype(np.float32)
    bo_eff = (bo + Wo @ bv).astype(np.float32)

    if not general:
        band0, band2, negm = _band_masks()

    in_maps = []
    for c in range(NCORES):
        bi, g = divmod(c, G)
        lo = SLOC * g - W
        halo = np.zeros((SH, D), dtype=np.float32)
        s0, s1 = max(lo, 0), min(lo + SH, S)
        halo[s0 - lo: s1 - lo] = hs[bi, s0:s1]
        hsT_c = np.ascontiguousarray(halo.T).astype(ml_dtypes.bfloat16)

        if general:
            m = _general_masks(am[bi], g)
        else:
            m = np.empty((NB, 2, P, 384), dtype=ml_dtypes.bfloat16)
            for n in range(NB):
                m[n, 0] = negm if (g == 0 and n == 0) else band0
                m[n, 1] = negm if (g == G - 1 and n == NB - 1) else band2

        in_maps.append(
            {
                "hsT": hsT_c,
                "wqT": wqT,
                "wkT": wkT,
                "wvT": wvT,
                "woT": woT,
                "bq": bq_s,
                "boe": bo_eff,
                "masks": m,
            }
        )
    return in_maps, general


def _run(inputs: dict, trace: bool = False):
    """Run the sharded kernel. Returns (full_output, BassKernelResults)."""
    from concourse.bass_utils import run_bass_kernel_spmd

    in_maps, general = _host_prep(**inputs)
    key = ("nc", general)
    if key not in _PROGRAM_CACHE:
        _PROGRAM_CACHE[key] = _build_program(general)
    nc = _PROGRAM_CACHE[key]

    res = run_bass_kernel_spmd(nc, in_maps, list(range(NCORES)), trace=trace)
    out = np.empty((B, S, D), dtype=np.float32)
    for c in range(NCORES):
        bi, g = divmod(c, G)
        out[bi, SLOC * g: SLOC * (g + 1), :] = res.results[c]["outT"].T
    return out, res


def kernel(**inputs) -> np.ndarray:
    out, _ = _run(inputs, trace=False)
    return out


# revision 5
# speedup vs baseline: 1.7874x; 1.1347x over previous
"""Longformer sliding-window self-attention (MBart variant) on 8 TRN2 cores.

Sequence-parallel sharding: core c -> batch c//4, queries
[1024*(c%4), 1024*(c%4+1)). Each core gets a halo'd, transposed,
bf16 slice of hidden states (1536 rows), computes Q/K/V projections,
banded attention, and the output projection, returning a [768, 1024]
fp32 transposed output slice. Host re-assembles [2, 4096, 768].

Design notes (all rewrites exact up to bf16 rounding of the inputs,
measured ~4e-3 max-rel on the fixed problem seed vs the 2e-2 gate):
  - hidden states + weights ship as bf16 (halves HBM traffic); query
    scale folded into Wq/bq, Wk bias cancels in softmax, Wv bias
    commutes through the probs (bo_eff = bo + Wo @ bv).
  - banded scores per (chunk n, head h) are trimmed: key tile 0 only
    covers queries 0:128, tile 5 only 128:256 (rest is out of band) ->
    1280 score columns in three [128,<=512] PSUM tiles.
  - band + sequence-edge masking is a post-exp multiply by a 0/1 bf16
    tile (DVE 2x mode) instead of a -1e9 pre-exp add; a non-zero
    attention_mask (general path) adds its per-key bias pre-exp via
    tensor_scalar (bias is constant along queries = per-partition).
  - context matmuls run TRANSPOSED: lhsT = exp tile [keys, queries],
    rhs = V augmented with a ones column [keys, 65] -> PSUM [queries,
    64 ctx + denominator]. Queries on partitions make the softmax
    denominator a per-partition scalar: one reciprocal [128,1] + one
    tensor_scalar_mul per (n, h, q-half). 128x128 bf16 DMA transposes
    rebuild the [D, seq] layout for the output projection.
  - emission order software-pipelines the whole kernel: V proj, then
    per head-pair j: Q(j), K(j), the 8 attention units (skewed so PE
    streams unit u+1 scores while unit u waits on exp), then that
    pair's DMA transposes. Keeps PE dense (HAM stays warm) and starts
    ScalarE exp work ~50 us earlier.
"""

import numpy as np
import ml_dtypes

# problem shapes (fixed by the task)
B, S, D, H = 2, 4096, 768, 12
DH = D // H            # 64
W = 256                # one-sided window == chunk size b
NEG = -1e9
NCORES = 8
G = 4                  # sequence groups per batch (8 cores / 2 batches)
SLOC = S // G          # 1024 queries per core
SH = SLOC + 2 * W      # 1536 halo'd rows per core
NB = SLOC // W         # 4 chunks per core
P = 128
DJ = D // P            # 6 tiles of 128 over the model dim

_PROGRAM_CACHE: dict = {}

# ctx matmul source columns in the expt slab per q-half: (key tile t, col)
# slab cols: t0(q0:128)->0:128, t1->128:384, t2->384:640, t3->640:896,
#            t4->896:1152, t5(q128:256)->1152:1280
_CTX_QH = [
    [(0, 0), (1, 128), (2, 384), (3, 640), (4, 896)],          # q 0:128
    [(1, 256), (2, 512), (3, 768), (4, 1024), (5, 1152)],      # q 128:256
]


def _build_program(general_mask: bool):
    import concourse.bacc as bacc
    import concourse.mybir as mybir
    import concourse.tile as tile
    from contextlib import ExitStack

    F32 = mybir.dt.float32
    BF = mybir.dt.bfloat16
    AF = mybir.ActivationFunctionType

    nc = bacc.Bacc("TRN2", target_bir_lowering=False, debug=False)

    hsT = nc.dram_tensor("hsT", [D, SH], BF, kind="ExternalInput")
    wqT = nc.dram_tensor("wqT", [D, D], BF, kind="ExternalInput")
    wkT = nc.dram_tensor("wkT", [D, D], BF, kind="ExternalInput")
    wvT = nc.dram_tensor("wvT", [D, D], BF, kind="ExternalInput")
    woT = nc.dram_tensor("woT", [D, D], BF, kind="ExternalInput")
    bq = nc.dram_tensor("bq", [P, DJ], F32, kind="ExternalInput")
    boe = nc.dram_tensor("boe", [P, DJ], F32, kind="ExternalInput")
    masks = nc.dram_tensor("masks", [NB, 2, P, 384], BF, kind="ExternalInput")
    if general_mask:
        gbias = nc.dram_tensor("gbias", [NB, P, 6], F32, kind="ExternalInput")
    outT = nc.dram_tensor("outT", [D, SLOC], F32, kind="ExternalOutput")

    with tile.TileContext(nc) as tc, ExitStack() as stack:
        const = stack.enter_context(tc.tile_pool(name="const", bufs=1))
        qt_p = stack.enter_context(tc.tile_pool(name="qt", bufs=1))
        kt_p = stack.enter_context(tc.tile_pool(name="kt", bufs=1))
        ct_p = stack.enter_context(tc.tile_pool(name="ct", bufs=1))
        va_p = stack.enter_context(tc.tile_pool(name="va", bufs=1))
        w_p = stack.enter_context(tc.tile_pool(name="wts", bufs=1))

        # ---- input DMAs, one per tensor, hidden states first ---------
        hs_stack = ExitStack()
        hs_p = hs_stack.enter_context(tc.tile_pool(name="hs", bufs=1))
        hs_all = hs_p.tile([P, DJ * SH], BF, tag="hs")
        hs_view = hs_all.rearrange("p (t c) -> p t c", c=SH)
        hsT_view = hsT.rearrange("(t p) c -> p t c", p=P)
        for ch in range(2):
            nc.sync.dma_start(
                out=hs_view[:, :, SH // 2 * ch: SH // 2 * (ch + 1)],
                in_=hsT_view[:, :, SH // 2 * ch: SH // 2 * (ch + 1)],
            )
        HS = [hs_all[:, SH * i: SH * (i + 1)] for i in range(DJ)]

        wv_all = w_p.tile([P, DJ * D], BF, tag="wv")
        nc.sync.dma_start(
            out=wv_all.rearrange("p (t c) -> p t c", c=D),
            in_=wvT.rearrange("(t p) c -> p t c", p=P),
        )
        WV = [wv_all[:, D * i: D * (i + 1)] for i in range(DJ)]
        wq_all = w_p.tile([P, DJ * D], BF, tag="wq")
        nc.sync.dma_start(
            out=wq_all.rearrange("p (t c) -> p t c", c=D),
            in_=wqT.rearrange("(t p) c -> p t c", p=P),
        )
        WQ = [wq_all[:, D * i: D * (i + 1)] for i in range(DJ)]
        wk_all = w_p.tile([P, DJ * D], BF, tag="wk")
        nc.sync.dma_start(
            out=wk_all.rearrange("p (t c) -> p t c", c=D),
            in_=wkT.rearrange("(t p) c -> p t c", p=P),
        )
        WK = [wk_all[:, D * i: D * (i + 1)] for i in range(DJ)]

        mask_sb = const.tile([P, NB * 2 * 384], BF, tag="masks")
        nc.sync.dma_start(
            out=mask_sb.rearrange("p (n s c) -> p n s c", s=2, c=384),
            in_=masks.rearrange("n s p c -> p n s c"),
        )
        bq_sb = const.tile([P, DJ], F32, tag="bq")
        nc.sync.dma_start(out=bq_sb[:], in_=bq[:, :])
        boe_sb = const.tile([P, DJ], F32, tag="boe")
        nc.sync.dma_start(out=boe_sb[:], in_=boe[:, :])
        if general_mask:
            gb_sb = const.tile([P, NB * 6], F32, tag="gbias")
            nc.sync.dma_start(
                out=gb_sb.rearrange("p (n c) -> p n c", c=6),
                in_=gbias.rearrange("n p c -> p n c"),
            )
        wo_all = w_p.tile([P, DJ * D], BF, tag="wo")
        nc.sync.dma_start(
            out=wo_all.rearrange("p (t c) -> p t c", c=D),
            in_=woT.rearrange("(t p) c -> p t c", p=P),
        )
        WO = [wo_all[:, D * i: D * (i + 1)] for i in range(DJ)]

        QT = [qt_p.tile([P, SLOC], BF, tag=f"qt{j}", name=f"qt{j}") for j in range(DJ)]
        KT = [kt_p.tile([P, SH], BF, tag=f"kt{j}", name=f"kt{j}") for j in range(DJ)]
        CT = [ct_p.tile([P, SLOC], BF, tag=f"ct{j}", name=f"ct{j}") for j in range(DJ)]
        VA = [va_p.tile([P, H * (DH + 1)], BF, tag=f"va{s}", name=f"va{s}")
              for s in range(SH // P)]
        for st in range(SH // P):
            nc.vector.memset(VA[st][:], 1.0)   # ones columns survive the V copy

        # ---------------- V projection --------------------------------
        with tc.tile_pool(name="ps2", bufs=4, space="PSUM") as ps2:
            for st in range(SH // P):
                for vh in range(2):
                    ps = ps2.tile([P, 384], F32, tag="ps2")
                    for i in range(DJ):
                        nc.tensor.matmul(
                            ps[:],
                            HS[i][:, P * st: P * (st + 1)],
                            WV[i][:, 384 * vh: 384 * (vh + 1)],
                            start=(i == 0),
                            stop=(i == DJ - 1),
                        )
                    view = VA[st].rearrange("p (h e) -> p h e", e=DH + 1)
                    nc.vector.tensor_copy(
                        view[:, 6 * vh: 6 * (vh + 1), 0:DH],
                        ps[:].rearrange("p (h e) -> p h e", e=DH),
                    )

        # ------- interleaved Q/K projections + banded attention -------
        with (
            tc.tile_pool(name="expp", bufs=3) as exp_p,
            tc.tile_pool(name="ctx", bufs=1) as ctx_p,
            tc.tile_pool(name="rcp", bufs=6) as rc_p,
            tc.tile_pool(name="pss", bufs=6, space="PSUM") as pss,
            tc.tile_pool(name="psc", bufs=2, space="PSUM") as psc,
        ):
            CX = [ctx_p.tile([P, H * DH], BF, tag=f"cx{u}", name=f"cx{u}")
                  for u in range(2 * NB)]

            def emit_scores(h, n):
                """Score matmuls + exp + band zeroing for one (head, chunk).
                Returns the bf16 exp slab [128 keys, 1280]."""
                jq, r0 = h // 2, DH * (h % 2)
                kb = W * n
                expt = exp_p.tile([P, 1280], BF, tag="e", name=f"e{n}_{h}")
                segs = [  # (group tiles, exp slab col, mask slot or None)
                    ([(0, 0, 128, 0), (1, 128, 384, 0)], 0, 0),
                    ([(2, 0, 256, 0), (3, 256, 512, 0)], 384, None),
                    ([(4, 0, 256, 0), (5, 256, 384, 128)], 896, 1),
                ]
                for gi, (tiles, ecol, mslot) in enumerate(segs):
                    gp = pss.tile([P, 512], F32, tag="s", name=f"s{gi}_{n}_{h}")
                    width = tiles[-1][2]
                    for t, c0, c1, qlo in tiles:
                        nc.tensor.matmul(
                            gp[:, c0:c1],
                            KT[jq][r0:r0 + DH, kb + P * t: kb + P * (t + 1)],
                            QT[jq][r0:r0 + DH, W * n + qlo: W * n + qlo + (c1 - c0)],
                            start=True,
                            stop=True,
                        )
                    if general_mask:
                        for t, c0, c1, qlo in tiles:
                            nc.vector.tensor_scalar_add(
                                gp[:, c0:c1], gp[:, c0:c1],
                                gb_sb[:, 6 * n + t: 6 * n + t + 1],
                            )
                    nc.scalar.activation(
                        expt[:, ecol: ecol + width], gp[:, 0:width], AF.Exp
                    )
                    if mslot is not None:
                        moff = (n * 2 + mslot) * 384
                        nc.vector.tensor_mul(
                            expt[:, ecol: ecol + 384],
                            expt[:, ecol: ecol + 384],
                            mask_sb[:, moff: moff + 384],
                        )
                return expt

            def emit_ctx(h, n, expt):
                """Transposed context + normalize into the CX slabs."""
                for qh in range(2):
                    cps = psc.tile([P, DH + 1], F32, tag="c", name=f"c{n}_{h}_{qh}")
                    segs = _CTX_QH[qh]
                    for si, (t, c0) in enumerate(segs):
                        nc.tensor.matmul(
                            cps[:],
                            expt[:, c0: c0 + P],
                            VA[2 * n + t][:, (DH + 1) * h: (DH + 1) * (h + 1)],
                            start=(si == 0),
                            stop=(si == len(segs) - 1),
                        )
                    rc = rc_p.tile([P, 1], F32, tag="rc", name=f"rc{n}_{h}_{qh}")
                    nc.vector.reciprocal(out=rc[:], in_=cps[:, DH: DH + 1])
                    nc.vector.tensor_scalar_mul(
                        CX[2 * n + qh][:, DH * h: DH * (h + 1)],
                        cps[:, 0:DH],
                        rc[:],
                    )

            pending = None      # (h, n, expt) with scores emitted, ctx not
            for j in range(DJ):
                # Q projection for head pair j
                for sp in range(SLOC // 512):
                    ps = pss.tile([P, 512], F32, tag="s", name=f"q{j}_{sp}")
                    for i in range(DJ):
                        nc.tensor.matmul(
                            ps[:],
                            WQ[i][:, P * j: P * (j + 1)],
                            HS[i][:, W + 512 * sp: W + 512 * (sp + 1)],
                            start=(i == 0),
                            stop=(i == DJ - 1),
                        )
                    nc.scalar.activation(
                        QT[j][:, 512 * sp: 512 * (sp + 1)],
                        ps[:],
                        AF.Identity,
                        bias=bq_sb[:, j: j + 1],
                    )
                # K projection for head pair j (bk cancels in softmax)
                for sp in range(SH // 512):
                    ps = pss.tile([P, 512], F32, tag="s", name=f"k{j}_{sp}")
                    for i in range(DJ):
                        nc.tensor.matmul(
                            ps[:],
                            WK[i][:, P * j: P * (j + 1)],
                            HS[i][:, 512 * sp: 512 * (sp + 1)],
                            start=(i == 0),
                            stop=(i == DJ - 1),
                        )
                    nc.vector.tensor_copy(
                        KT[j][:, 512 * sp: 512 * (sp + 1)], ps[:]
                    )
                # attention units, skew-1 pipelined
                for h in (2 * j, 2 * j + 1):
                    for n in range(NB):
                        expt = emit_scores(h, n)
                        if pending is not None:
                            ph, pn, pexpt = pending
                            emit_ctx(ph, pn, pexpt)
                            if ph % 2 == 1 and pn == NB - 1:
                                pj = ph // 2
                                for n2 in range(NB):
                                    for qh in range(2):
                                        nc.sync.dma_start_transpose(
                                            out=CT[pj][:, W * n2 + P * qh:
                                                       W * n2 + P * (qh + 1)],
                                            in_=CX[2 * n2 + qh][:, P * pj:
                                                                P * (pj + 1)],
                                        )
                        pending = (h, n, expt)
            ph, pn, pexpt = pending
            emit_ctx(ph, pn, pexpt)
            for n2 in range(NB):
                for qh in range(2):
                    nc.sync.dma_start_transpose(
                        out=CT[DJ - 1][:, W * n2 + P * qh: W * n2 + P * (qh + 1)],
                        in_=CX[2 * n2 + qh][:, P * (DJ - 1): P * DJ],
                    )
        hs_stack.close()

        # ---------------- output projection ---------------------------
        with (
            tc.tile_pool(name="ob", bufs=3) as ob_p,
            tc.tile_pool(name="ps3", bufs=4, space="PSUM") as ps3,
        ):
            for j in range(DJ):
                for sp in range(SLOC // 512):
                    ps = ps3.tile([P, 512], F32, tag="ps3")
                    for i in range(DJ):
                        nc.tensor.matmul(
                            ps[:],
                            WO[i][:, P * j: P * (j + 1)],
                            CT[i][:, 512 * sp: 512 * (sp + 1)],
                            start=(i == 0),
                            stop=(i == DJ - 1),
                        )
                    osb = ob_p.tile([P, 512], F32, tag="ob")
                    nc.scalar.activation(
                        osb[:], ps[:], AF.Identity, bias=boe_sb[:, j: j + 1]
                    )
                    nc.sync.dma_start(
                        out=outT[P * j: P * (j + 1), 512 * sp: 512 * (sp + 1)],
                        in_=osb[:],
                    )

    nc.compile()
    return nc


def _band_masks01():
    """Multiplicative band masks, bf16 0/1, [128, 384] each."""
    p = np.arange(P)[:, None]
    c = np.arange(384)[None, :]
    # G0: cols 0:128 = t0 (q = c, valid q <= p); 128:384 = t1 (q = c-128,
    # valid q <= p + 128)
    q0 = np.where(c < 128, c, c - 128)
    v0 = np.where(c < 128, q0 <= p, q0 <= p + 128)
    band0 = v0.astype(ml_dtypes.bfloat16)
    # G2: cols 0:256 = t4 (q = c, valid q >= p); 256:384 = t5 (q = c-128,
    # valid q >= p + 128)
    q2 = np.where(c < 256, c, c - 128)
    v2 = np.where(c < 256, q2 >= p, q2 >= p + 128)
    band2 = v2.astype(ml_dtypes.bfloat16)
    zeros = np.zeros((P, 384), dtype=ml_dtypes.bfloat16)
    return band0, band2, zeros


def _host_prep(hidden_states, attention_mask, Wq, bq, Wk, bk, Wv, bv, Wo, bo):
    """Build per-core input maps. Returns (in_maps, general_mask)."""
    hs = np.asarray(hidden_states, dtype=np.float32)
    am = np.asarray(attention_mask, dtype=np.float32)
    Wq = np.asarray(Wq, dtype=np.float32)
    Wk = np.asarray(Wk, dtype=np.float32)
    Wv = np.asarray(Wv, dtype=np.float32)
    Wo = np.asarray(Wo, dtype=np.float32)
    bq = np.asarray(bq, dtype=np.float32)
    bv = np.asarray(bv, dtype=np.float32)
    bo = np.asarray(bo, dtype=np.float32)

    general = bool(np.any(am != 0.0))
    scale = 1.0 / np.sqrt(np.float32(DH))

    wqT = np.ascontiguousarray(Wq.T * scale).astype(ml_dtypes.bfloat16)
    wkT = np.ascontiguousarray(Wk.T).astype(ml_dtypes.bfloat16)
    wvT = np.ascontiguousarray(Wv.T).astype(ml_dtypes.bfloat16)
    woT = np.ascontiguousarray(Wo.T).astype(ml_dtypes.bfloat16)
    bq_pt = np.ascontiguousarray((bq * scale).reshape(DJ, P).T).astype(np.float32)
    bo_eff = np.ascontiguousarray(
        (bo + Wo @ bv).reshape(DJ, P).T
    ).astype(np.float32)

    band0, band2, zeros = _band_masks01()

    in_maps = []
    for c in range(NCORES):
        bi, g = divmod(c, G)
        lo = SLOC * g - W
        halo = np.zeros((SH, D), dtype=np.float32)
        s0, s1 = max(lo, 0), min(lo + SH, S)
        halo[s0 - lo: s1 - lo] = hs[bi, s0:s1]
        hsT_c = np.ascontiguousarray(halo.T).astype(ml_dtypes.bfloat16)

        m = np.empty((NB, 2, P, 384), dtype=ml_dtypes.bfloat16)
        for n in range(NB):
            m[n, 0] = zeros if (g == 0 and n == 0) else band0
            m[n, 1] = zeros if (g == G - 1 and n == NB - 1) else band2

        in_map = {
            "hsT": hsT_c,
            "wqT": wqT,
            "wkT": wkT,
            "wvT": wvT,
            "woT": woT,
            "bq": bq_pt,
            "boe": bo_eff,
            "masks": m,
        }
        if general:
            # per-key additive bias, constant along queries: [NB, P, 6]
            gb = np.zeros((NB, P, 6), dtype=np.float32)
            p_idx = np.arange(P)[:, None]
            for n in range(NB):
                kglob = SLOC * g - W + W * n + np.arange(6)[None, :] * P + p_idx
                inb = (kglob >= 0) & (kglob < S)
                gb[n] = np.where(inb, -am[bi, np.clip(kglob, 0, S - 1)], 0.0)
            in_map["gbias"] = gb
        in_maps.append(in_map)
    return in_maps, general


def _run(inputs: dict, trace: bool = False):
    """Run the sharded kernel. Returns (full_output, BassKernelResults)."""
    from concourse.bass_utils import run_bass_kernel_spmd

    in_maps, general = _host_prep(**inputs)
    key = ("nc", general)
    if key not in _PROGRAM_CACHE:
        _PROGRAM_CACHE[key] = _build_program(general)
    nc = _PROGRAM_CACHE[key]

    res = run_bass_kernel_spmd(nc, in_maps, list(range(NCORES)), trace=trace)
    out = np.empty((B, S, D), dtype=np.float32)
    for c in range(NCORES):
        bi, g = divmod(c, G)
        out[bi, SLOC * g: SLOC * (g + 1), :] = res.results[c]["outT"].T
    return out, res


def kernel(**inputs) -> np.ndarray:
    out, _ = _run(inputs, trace=False)
    return out
